# revision 73
# baseline (speedup 1.0000x reference)
"""Trainium2 Bass kernel for the CudaNorm FastWeight DPFP transformer layer.

Sharding: batch (8) across the 8 cores; each core runs its batch's full layer.

v5: software pipeline reordered so the mask/gram chain (prepB2a) for chunk c+1
runs a full period ahead of its consumer (prepB2b/gram_back), per-half
roll->transpose interleave with split G tiles, W-update in 4x4-head groups
with split W tiles, psKD double-buffered, masks split Pool/DVE, unnormalized-K
trick, PE ones-matmul feature sums, LN center/scale on Act (ln_g/ln_b on
host).
"""
import os
import numpy as np
import ml_dtypes

import concourse.bass as bass
import concourse.mybir as mybir
from concourse.bass_utils import run_bass_kernel_spmd
from concourse.tile import TileContext
from concourse.vector_clock import ScopedClock, VectorClock
from contextlib import ExitStack

F32 = mybir.dt.float32
BF16 = mybir.dt.bfloat16
F8 = mybir.dt.float8e4
DR = mybir.MatmulPerfMode.DoubleRow
AF = mybir.ActivationFunctionType
OP = mybir.AluOpType
AX = mybir.AxisListType

SLEN, BSZ, DM = 2048, 8, 1024
NH, DH, NROLL = 16, 64, 2
D = 2 * NROLL * DH            # 256 feature dim
C = 128                       # chunk length
NCH = SLEN // C               # 16 chunks
EPS, LN_EPS = 1e-5, 1e-5
SCALE = 1.0 / float(np.sqrt(DH))
NHORN = 1                     # Horner/Neumann solve iterations
MASK_DVE = {6, 7}             # gram pr indices whose masks run on DVE

# ---------------------------------------------------------------- tile ctx
MAXW = 2


class PatchedTileContext(TileContext):
    """Work around walrus TPB sync-command limits: each instruction carries at
    most 2 sync commands (waits+updates); hoist excess waits onto preceding
    same-engine NoOps (1 wait each), and emit the kernel-tail drain's waits
    one-per-nop on SP."""

    def _lower_ordered_insts(self, ordered):
        for bb_name in list(ordered.keys()):
            new = []
            for inst in ordered[bb_name]:
                si = inst.sync_info
                nupd = len(si.on_update) if si is not None and si.on_update else 0
                maxw = max(0, MAXW - nupd)
                if si is not None and si.on_wait and len(si.on_wait) > maxw:
                    waits = list(si.on_wait)
                    excess = waits if maxw == 0 else waits[:-maxw]
                    keep = [] if maxw == 0 else waits[-maxw:]
                    for w in excess:
                        nop = mybir.InstNoOp(
                            name=self.nc.get_next_instruction_name(),
                            engine=inst.engine, ins=[], outs=[])
                        nop.sync_info = mybir.SyncInfo(on_wait=[w], on_update=[])
                        new.append(nop)
                    inst.sync_info = mybir.SyncInfo(
                        on_wait=keep, on_update=list(si.on_update or []))
                new.append(inst)
            ordered[bb_name] = new
        return super()._lower_ordered_insts(ordered)

    def _drain_and_barrier(self, tick_clock, wait_clock):
        gc = tick_clock.global_clock
        n = len(gc)
        for p in range(n):
            if gc[p] > 0:
                vc = VectorClock([gc[i] if i == p else 0 for i in range(n)])
                nop = self.nc.sync.nop(nofuse=True)
                wait_clock.add_sem_waits(nop.ins, ScopedClock({None: vc}))
        self.nc.sync.drain()
        self.nc.all_engine_barrier()
        assert self.sems is not None
        popped = self.nc._tile_sem_poison_stack.pop()
        assert popped is self._sem_poison
        self.nc.clear_and_free_semaphores(list(self.sems.allocated().values()))
        self.nc.all_engine_barrier()


# ---------------------------------------------------------------- program
def build_program(n_chunks=NCH):
    nc = bass.Bass()
    d_h8 = nc.declare_dram_parameter("h8", [DM, SLEN], F8, isOutput=False)
    d_dh8 = nc.declare_dram_parameter("dh8", [DM, SLEN], F8, isOutput=False)
    d_hres = nc.declare_dram_parameter("hres", [SLEN, DM + 8], BF16,
                                       isOutput=False)
    d_w8 = nc.declare_dram_parameter("w8", [DM, 3088], F8, isOutput=False)
    d_dw8 = nc.declare_dram_parameter("dw8", [DM, 3088], F8, isOutput=False)
    d_wo8 = nc.declare_dram_parameter("wo8", [DM, DM], F8, isOutput=False)
    d_woRS = nc.declare_dram_parameter("woRS", [128, 8], BF16, isOutput=False)
    d_mUI = nc.declare_dram_parameter("maskUI", [128, 128], BF16, isOutput=False)
    d_mUS = nc.declare_dram_parameter("maskUS", [128, 128], BF16, isOutput=False)
    d_out = nc.declare_dram_parameter("out", [SLEN, DM], BF16, isOutput=True)

    with PatchedTileContext(nc) as tc, ExitStack() as ctx:
        P = lambda name, bufs, **kw: ctx.enter_context(
            tc.tile_pool(name=name, bufs=bufs, **kw))
        const = P("const", 1)
        state = P("state", 1)
        hts_p = P("hts", 2)
        xp_p = P("xp", 2)
        f_p = P("f", 2)
        G_p = P("G", 2)
        V_p = P("V", 2)
        t1_p = P("t1", 2)
        cols_p = P("cols", 3)
        sa_p = P("sa", 3)
        sh_p = P("sh", 2)
        nt_p = P("nt", 2)
        mg_p = P("mg", 3)
        oc_p = P("oc", 2)
        oT_p = P("oT", 1)
        hr_p = P("hr", 2)
        x_p = P("x", 2)
        psP_p = P("psP", 2, space="PSUM")
        psG_p = P("psG", 2, space="PSUM")
        psS_p = P("psS", 2, space="PSUM")
        psK_p = P("psK", 2, space="PSUM")

        # ---- constants
        t_mUI = const.tile([128, 128], BF16, tag="mUI", name="mUI")
        t_mUS = const.tile([128, 128], BF16, tag="mUS", name="mUS")
        t_ones = const.tile([128, 1], BF16, tag="ones", name="ones")
        nc.vector.memset(t_ones[:], 1.0)
        # fp8 DoubleRow weight tiles: [128, 2 dm-blocks x 3088]
        t_w8 = [const.tile([128, 2 * 3088], F8, tag=f"w8{m}", name=f"w8{m}")
                for m in range(4)]
        t_dw8 = [const.tile([128, 2 * 3088], F8, tag=f"dw8{m}", name=f"dw8{m}")
                 for m in range(4)]
        # Wo in fp8 pair-interleaved DoubleRow layout (values are 64*W_o);
        # row (m, p, e) holds W_o[:, 256m + 2p + e]
        t_wo8 = [const.tile([128, 2 * DM], F8, tag=f"wo8{m}", name=f"wo8{m}")
                 for m in range(4)]
        t_woRS = const.tile([128, 8], BF16, tag="woRS", name="woRS")

        def load_consts():
            for m in range(4):
                for e in range(2):
                    mc = 2 * m + e
                    nc.sync.dma_start(
                        t_w8[m][:, e * 3088:(e + 1) * 3088],
                        d_w8[mc * 128:(mc + 1) * 128, :])
                    nc.sync.dma_start(
                        t_dw8[m][:, e * 3088:(e + 1) * 3088],
                        d_dw8[mc * 128:(mc + 1) * 128, :])
            nc.sync.dma_start(t_mUI[:], d_mUI[:])
            nc.sync.dma_start(t_mUS[:], d_mUS[:])
            nc.sync.dma_start(t_woRS[:], d_woRS[:])
            for m in range(4):
                for e in range(2):
                    ic = 2 * m + e
                    nc.sync.dma_start(t_wo8[m][:, e * DM:(e + 1) * DM],
                                      d_wo8[ic * 128:(ic + 1) * 128, :])

        # ---- state (W accumulates directly in bf16; increments are bf16
        # products so fp32 mastering gains nothing - verified in proto)
        t_Wh = [state.tile([128, 8 * 128], BF16, tag=f"Wb{hf}", name=f"Wb{hf}")
                for hf in range(2)]
        for hf in range(2):
            nc.vector.memset(t_Wh[hf][:], 0.0)
        t_r32 = state.tile([128, 32], F32, tag="r32", name="r32")
        nc.vector.memset(t_r32[:], 0.0)
        t_rb = state.tile([128, 32], BF16, tag="rb", name="rb")
        nc.vector.memset(t_rb[:], 0.0)
        t_Y = state.tile([128, NH * DH], BF16, tag="Y", name="Y")

        def Wb(hd):
            return t_Wh[hd // 8], (hd % 8) * 128

        H = {}

        def prepA_dma(c):
            cs = slice(c * 128, (c + 1) * 128)
            hts = hts_p.tile([128, 1024], F8, tag="hts", name="hts")
            nc.sync.dma_start(
                hts[:].rearrange("p (mc t) -> p mc t", t=128),
                d_h8[:, cs].rearrange("(mc p) t -> p mc t", p=128))
            dhts = hts_p.tile([128, 1024], F8, tag="dhts", name="dhts")
            nc.sync.dma_start(
                dhts[:].rearrange("p (mc t) -> p mc t", t=128),
                d_dh8[:, cs].rearrange("(mc p) t -> p mc t", p=128))
            hr = hr_p.tile([128, DM + 8], BF16, tag="hr", name="hr")
            nc.sync.dma_start(hr[:], d_hres[cs, :])
            return hts, dhts, hr

        def proj_qkvb(ps, hts, dhts, lo, hi):
            """ps += h @ W_qkvb[:, lo:hi] via 3-term fp8 DoubleRow matmuls."""
            hv = hts[:].rearrange("p (mc t) -> p mc t", t=128)
            dhv = dhts[:].rearrange("p (mc t) -> p mc t", t=128)
            k, terms = 0, [(hv, t_w8), (hv, t_dw8), (dhv, t_w8)]
            for ht, tw in terms:
                for m in range(4):
                    wv = tw[m][:].rearrange("p (e w) -> p e w", w=3088)
                    nc.tensor.matmul(ps[:], ht[:, 2 * m:2 * m + 2, :],
                                     wv[:, :, lo:hi],
                                     start=(k == 0), stop=(k == 11),
                                     perf_mode=DR)
                    k += 1

        def make_prepA(c):
            """Incremental projection emitter: emit(n) lays down n og-groups
            (12 DR matmuls + activation each); og 6 is the sigmoid b block."""
            hts, dhts, hr = prepA_dma(c)
            xpC = xp_p.tile([128, 4096], BF16, tag="xpC", name="xpC")
            xr = xpC[:].rearrange("p (h w) -> p h w", w=256)
            Vall = V_p.tile([128, 1024], BF16, tag="V", name="V")
            sig = cols_p.tile([128, NH], F32, tag="sig", name="sig")
            H[c] = dict(xpC=xpC, Vall=Vall, sig=sig, hr=hr)
            st = {"og": 0}

            def emit(n):
                for _ in range(n):
                    og = st["og"]
                    if og >= 7:
                        return
                    st["og"] += 1
                    if og < 6:
                        ps = psP_p.tile([128, 512], F32, tag="pp", name="pp")
                        proj_qkvb(ps, hts, dhts, og * 512, (og + 1) * 512)
                        psr = ps[:].rearrange("p (h v) -> p h v", v=64)
                        if og < 4:  # K (og 0,1) / Q (og 2,3)
                            o = 128 if og < 2 else 0
                            hs = slice((og % 2) * 8, (og % 2) * 8 + 8)
                            nc.scalar.activation(xr[:, hs, o:o + 64], psr,
                                                 AF.Relu)
                            nc.scalar.activation(xr[:, hs, o + 64:o + 128],
                                                 psr, AF.Relu, scale=-1.0)
                        else:       # V
                            gv = og - 4
                            nc.scalar.copy(Vall[:, gv * 512:(gv + 1) * 512],
                                           ps[:])
                    else:
                        psb = psP_p.tile([128, NH], F32, tag="pp", name="pp")
                        proj_qkvb(psb, hts, dhts, 3072, 3088)
                        nc.scalar.activation(sig[:], psb[:], AF.Sigmoid)
            return emit

        def prepB1_half(c, hf):
            h = H[c]
            if hf == 0:
                # f tiles: per dc, per head [Q-roll 128 | K-roll 128]; K raw
                h["f"] = [f_p.tile([128, 4096], BF16, tag=f"f{dc}",
                                   name=f"f{dc}") for dc in range(2)]
                # G[dc][hf]: 16 blocks [128f x 128t] for heads hf*8..hf*8+7
                h["G"] = [[G_p.tile([128, 2048], BF16, tag=f"G{dc}{g}",
                                    name=f"G{dc}{g}") for g in range(2)]
                          for dc in range(2)]
            f, G = h["f"], h["G"]
            fr = [f[dc][:].rearrange("p (h w) -> p h w", w=256)
                  for dc in range(2)]
            xr = h["xpC"][:].rearrange("p (h w) -> p h w", w=256)
            hs = slice(hf * 8, hf * 8 + 8)
            for dc in range(2):
                rl = dc + 1
                for o in (0, 128):  # Q block, K block per head
                    nc.vector.tensor_mul(fr[dc][:, hs, o + rl:o + 128],
                                         xr[:, hs, o + rl:o + 128],
                                         xr[:, hs, o:o + 128 - rl])
                    nc.vector.tensor_mul(fr[dc][:, hs, o:o + rl],
                                         xr[:, hs, o:o + rl],
                                         xr[:, hs, o + 128 - rl:o + 128])
            for dc in range(2):
                nc.sync.dma_start_transpose(
                    G[dc][hf][:].rearrange("p (b l) -> p b l", l=128),
                    f[dc][:, hf * 2048:(hf + 1) * 2048])

        def Gb(h, dc, hd):
            """(tile, base offset) of head hd's [Q128|K128] block pair."""
            return h["G"][dc][hd // 8], (hd % 8) * 256

        def prepB2a_half(c, hf):
            h = H[c]
            if hf == 0:
                # psKD bank: [kd 16|dn 16|sumQ 16|sumK 16|rcols 32|xsum 1]
                h["psKD"] = psK_p.tile([128, 97], F32, tag="kdps", name="kdps")
                h["Shat"] = sh_p.tile([128, NH * 128], BF16, tag="sh",
                                      name="sh")
                h["NT"] = nt_p.tile([128, NH * 128], BF16, tag="nt", name="nt")
            psKD, Shat, NT = h["psKD"], h["Shat"], h["NT"]

            def gram_front(pr):
                ps = psG_p.tile([128, 512], F32, tag="gg", name="gg")
                for e in range(2):
                    hd = pr * 2 + e
                    Gt0, b0 = Gb(h, 0, hd)
                    Gt1, b1 = Gb(h, 1, hd)
                    nc.tensor.matmul(ps[:, e * 256:e * 256 + 256],
                                     Gt0[:, b0 + 128:b0 + 256],
                                     Gt0[:, b0:b0 + 256],
                                     start=True, stop=False)
                    nc.tensor.matmul(ps[:, e * 256:e * 256 + 256],
                                     Gt1[:, b1 + 128:b1 + 256],
                                     Gt1[:, b1:b1 + 256],
                                     start=False, stop=True)
                sa = sa_p.tile([128, 512], BF16, tag="sa", name="sa")
                nc.scalar.copy(sa[:], ps[:])
                sar = sa[:].rearrange("p (e w) -> p e w", w=256)
                eng = nc.vector if pr in MASK_DVE else nc.gpsimd
                eng.tensor_mul(
                    Shat[:, pr * 256:(pr + 1) * 256].rearrange(
                        "p (e w) -> p e w", w=128),
                    sar[:, :, 0:128],
                    t_mUI[:].unsqueeze(1).broadcast_to((128, 2, 128)))
                eng.tensor_mul(
                    NT[:, pr * 256:(pr + 1) * 256].rearrange(
                        "p (e w) -> p e w", w=128),
                    sar[:, :, 128:256],
                    t_mUS[:].unsqueeze(1).broadcast_to((128, 2, 128)))

            # feature sums via PE ones-matmuls on raw G blocks
            for hd in range(hf * 8, hf * 8 + 8):
                for dc in range(2):
                    Gt, b = Gb(h, dc, hd)
                    nc.tensor.matmul(psKD[:, 32 + hd:33 + hd],
                                     Gt[:, b:b + 128], t_ones[:],
                                     start=(dc == 0), stop=(dc == 1))
                for dc in range(2):
                    Gt, b = Gb(h, dc, hd)
                    nc.tensor.matmul(psKD[:, 48 + hd:49 + hd],
                                     Gt[:, b + 128:b + 256], t_ones[:],
                                     start=(dc == 0), stop=(dc == 1))
            for pr in range(hf * 4, hf * 4 + 4):
                gram_front(pr)

        def prepB2a_fin(c):
            h = H[c]
            psKD = h["psKD"]
            frecQ = cols_p.tile([128, NH], F32, tag="frecQ", name="frecQ")
            nc.vector.reciprocal(frecQ[:], psKD[:, 32:48])
            cKf = cols_p.tile([128, NH], F32, tag="cKf", name="cKf")
            nc.vector.reciprocal(cKf[:], psKD[:, 48:64])
            cKb = cols_p.tile([128, NH], BF16, tag="cKb", name="cKb")
            nc.scalar.copy(cKb[:], cKf[:])
            cK2 = cols_p.tile([128, NH], F32, tag="cK2", name="cK2")
            nc.vector.tensor_mul(cK2[:], cKf[:], cKf[:])
            fQS = cols_p.tile([128, NH], F32, tag="fQS", name="fQS")
            nc.vector.tensor_scalar_mul(fQS[:], frecQ[:], SCALE)
            h.update(frecQ=frecQ, cKf=cKf, cKb=cKb, cK2=cK2, fQS=fQS)

        def prepB2b(c):
            h = H[c]
            psKD, cKb, cKf, frecQ = h["psKD"], h["cKb"], h["cKf"], h["frecQ"]
            Shat, NT = h["Shat"], h["NT"]

            def gram_back(pr):
                for e in range(2):
                    hd = pr * 2 + e
                    for dc in range(2):
                        Gt, b = Gb(h, dc, hd)
                        nc.tensor.matmul(psKD[:, hd:hd + 1],
                                         Gt[:, b + 128:b + 256],
                                         t_rb[:, dc * 16 + hd:dc * 16 + hd + 1],
                                         start=(dc == 0), stop=False)
                    nc.tensor.matmul(psKD[:, hd:hd + 1],
                                     NT[:, hd * 128:(hd + 1) * 128],
                                     cKb[:, hd:hd + 1], start=False, stop=True)
                    for dc in range(2):
                        Gt, b = Gb(h, dc, hd)
                        nc.tensor.matmul(psKD[:, 16 + hd:17 + hd],
                                         Gt[:, b:b + 128],
                                         t_rb[:, dc * 16 + hd:dc * 16 + hd + 1],
                                         start=(dc == 0), stop=False)
                    nc.tensor.matmul(psKD[:, 16 + hd:17 + hd],
                                     Shat[:, hd * 128:(hd + 1) * 128],
                                     cKb[:, hd:hd + 1], start=False, stop=True)

            for pr in range(NH // 2):
                gram_back(pr)
            kd = cols_p.tile([128, NH], F32, tag="kd", name="kd")
            nc.vector.tensor_mul(kd[:], psKD[:, 0:16], cKf[:])
            if c == 0:
                nc.vector.memset(kd[0:1, :], 1.0)

            # chunk-level columns
            ceps = cols_p.tile([128, NH], F32, tag="ceps", name="ceps")
            nc.vector.tensor_scalar_add(ceps[:], kd[:], EPS)
            cc = cols_p.tile([128, NH], F32, tag="cc", name="cc")
            nc.vector.reciprocal(cc[:], ceps[:])
            t0 = cols_p.tile([128, NH], F32, tag="t0", name="t0")
            nc.vector.tensor_mul(t0[:], kd[:], cc[:])
            cb = cols_p.tile([128, NH], F32, tag="cb", name="cb")
            nc.vector.tensor_mul(cb[:], t0[:], h["sig"][:])
            cbK = cols_p.tile([128, NH], F32, tag="cbK", name="cbK")
            nc.vector.tensor_mul(cbK[:], cb[:], cKf[:])
            cbc = cols_p.tile([128, NH], F32, tag="cbc", name="cbc")
            nc.vector.tensor_mul(cbc[:], cb[:], cc[:])
            cbc2 = cols_p.tile([128, NH], F32, tag="cbc2", name="cbc2")
            nc.vector.tensor_mul(cbc2[:], cbc[:], h["cK2"][:])
            dnm = cols_p.tile([128, NH], F32, tag="dnm", name="dnm")
            nc.vector.tensor_mul(dnm[:], psKD[:, 16:32], frecQ[:])
            dne = cols_p.tile([128, NH], F32, tag="dne", name="dne")
            nc.vector.tensor_scalar_add(dne[:], dnm[:], EPS)
            dnr = cols_p.tile([128, NH], F32, tag="dnr", name="dnr")
            nc.vector.reciprocal(dnr[:], dne[:])
            dnrS = cols_p.tile([128, NH], F32, tag="dnrS", name="dnrS")
            nc.vector.tensor_mul(dnrS[:], dnr[:], h["fQS"][:])
            # t1-hat = (cb*cK) * V, broadcast mul on Pool (SBUF-only operands)
            t1 = t1_p.tile([128, 1024], BF16, tag="t1", name="t1")
            nc.gpsimd.tensor_mul(
                t1[:].rearrange("p (h v) -> p h v", v=64),
                h["Vall"][:].rearrange("p (h v) -> p h v", v=64),
                cbK[:].unsqueeze(2).broadcast_to((128, NH, 64)))
            h.update(cbc2=cbc2, dnrS=dnrS, t1=t1)

        def bscale(out, ps, colvec, g):
            """out (bf16) = psum [128,512] * per-head column broadcast."""
            nc.vector.tensor_tensor(
                out.rearrange("p (h v) -> p h v", v=64),
                ps[:].rearrange("p (h v) -> p h v", v=64),
                colvec[:, g * 8:(g + 1) * 8].unsqueeze(2).broadcast_to((128, 8, 64)),
                OP.mult)

        def scan_core(c, emitA, nxt):
            h = H[c]
            f, cbc2, dnrS = h["f"], h["cbc2"], h["dnrS"]
            cKb, psKD = h["cKb"], h["psKD"]
            emitA(2)
            if nxt:
                prepB1_half(c + 1, 0)
            # ---- KW + X0 = t1 - cbc2*KW (in place over t1), subs on Pool
            X0 = h["t1"]
            for g in range(2):
                ps = psS_p.tile([128, 512], F32, tag="ss", name="ss")
                for j in range(8):
                    hd = g * 8 + j
                    Gt, b = Gb(h, 0, hd)
                    Gt1, b1 = Gb(h, 1, hd)
                    Wt, wb = Wb(hd)
                    nc.tensor.matmul(ps[:, j * 64:(j + 1) * 64],
                                     Gt[:, b + 128:b + 256],
                                     Wt[:, wb:wb + 64],
                                     start=True, stop=False)
                    nc.tensor.matmul(ps[:, j * 64:(j + 1) * 64],
                                     Gt1[:, b1 + 128:b1 + 256],
                                     Wt[:, wb + 64:wb + 128],
                                     start=False, stop=True)
                mg = mg_p.tile([128, 512], BF16, tag="mg", name="mg")
                bscale(mg[:], ps, cbc2, g)
                nc.gpsimd.tensor_sub(X0[:, g * 512:(g + 1) * 512],
                                     X0[:, g * 512:(g + 1) * 512], mg[:])
            emitA(2)
            if nxt:
                prepB1_half(c + 1, 1)
            # ---- Horner iterations: Z <- X0 - cbc2*(NT^T Z); last writes t_Y
            Zin = X0
            for it in range(NHORN):
                for g in range(2):
                    ps = psS_p.tile([128, 512], F32, tag="ss", name="ss")
                    for j in range(8):
                        hd = g * 8 + j
                        nc.tensor.matmul(ps[:, j * 64:(j + 1) * 64],
                                         h["NT"][:, hd * 128:(hd + 1) * 128],
                                         Zin[:, hd * 64:(hd + 1) * 64],
                                         start=True, stop=True)
                    mg = mg_p.tile([128, 512], BF16, tag="mg", name="mg")
                    bscale(mg[:], ps, cbc2, g)
                    nc.gpsimd.tensor_sub(t_Y[:, g * 512:(g + 1) * 512],
                                         X0[:, g * 512:(g + 1) * 512], mg[:])
                Zin = t_Y
            emitA(1)
            # ---- outc = (QW + tril_incl(S1) Yhat) * dnrS
            outc = oc_p.tile([128, 1024], BF16, tag="oc", name="oc")
            for g in range(2):
                ps = psS_p.tile([128, 512], F32, tag="ss", name="ss")
                for j in range(8):
                    hd = g * 8 + j
                    Wt, wb = Wb(hd)
                    for dc in range(2):
                        Gt, b = Gb(h, dc, hd)
                        nc.tensor.matmul(ps[:, j * 64:(j + 1) * 64],
                                         Gt[:, b:b + 128],
                                         Wt[:, wb + dc * 64:wb + dc * 64 + 64],
                                         start=(dc == 0), stop=False)
                    nc.tensor.matmul(ps[:, j * 64:(j + 1) * 64],
                                     h["Shat"][:, hd * 128:(hd + 1) * 128],
                                     t_Y[:, hd * 64:(hd + 1) * 64],
                                     start=False, stop=True)
                bscale(outc[:, g * 512:(g + 1) * 512], ps, dnrS, g)
            # ---- transpose outc -> outT [i, t] via one DMA, then fp8 cast
            outT = oT_p.tile([128, 1024], BF16, tag="oT", name="oT")
            nc.sync.dma_start_transpose(
                outT[:].rearrange("p (b l) -> p b l", l=128), outc[:])
            oT8 = oT_p.tile([128, 1024], F8, tag="oT8", name="oT8")
            nc.scalar.copy(oT8[:], outT[:])
            h["outT"] = oT8
            emitA(1)
            if nxt:
                prepB2a_half(c + 1, 0)
            # ---- W state update: Wm += Kraw^T Yhat; r += Kraw^T cK
            for q in range(4):
                hds = range(q * 4, q * 4 + 4)
                ps = psS_p.tile([128, 512], F32, tag="ss", name="ss")
                for j, hd in enumerate(hds):
                    for dc in range(2):
                        nc.tensor.matmul(
                            ps[:, j * 128 + dc * 64:j * 128 + dc * 64 + 64],
                            f[dc][:, hd * 256 + 128:hd * 256 + 256],
                            t_Y[:, hd * 64:(hd + 1) * 64],
                            start=True, stop=True)
                        nc.tensor.matmul(
                            psKD[:, 64 + dc * 16 + hd:65 + dc * 16 + hd],
                            f[dc][:, hd * 256 + 128:hd * 256 + 256],
                            cKb[:, hd:hd + 1], start=True, stop=True)
                Wt, wb = Wb(q * 4)
                nc.vector.tensor_add(Wt[:, wb:wb + 512], ps[:],
                                     Wt[:, wb:wb + 512])
            nc.vector.tensor_add(t_r32[:], psKD[:, 64:96], t_r32[:])
            nc.scalar.copy(t_rb[:], t_r32[:])
            emitA(1)
            if nxt:
                prepB2a_half(c + 1, 1)
                prepB2a_fin(c + 1)

        def scan_tail(c):
            cs = slice(c * 128, (c + 1) * 128)
            h = H.pop(c)
            outT = h["outT"]
            # ---- output projection + residual + LN (center/scale via Act)
            x = x_p.tile([128, DM], BF16, tag="x", name="x")
            oTv = outT[:].rearrange("p (ic t) -> p ic t", t=128)
            for og in range(2):
                ps = psP_p.tile([128, 512], F32, tag="pp", name="pp")
                for m in range(4):
                    wv = t_wo8[m][:].rearrange("p (e w) -> p e w", w=DM)
                    nc.tensor.matmul(ps[:], oTv[:, 2 * m:2 * m + 2, :],
                                     wv[:, :, og * 512:(og + 1) * 512],
                                     start=(m == 0), stop=(m == 3),
                                     perf_mode=DR)
                # psum holds 64*attn (wo8 stores 64*W_o); rescale in the add
                nc.vector.scalar_tensor_tensor(
                    x[:, og * 512:(og + 1) * 512], ps[:], 1.0 / 64.0,
                    h["hr"][:, og * 512:(og + 1) * 512], OP.mult, OP.add)
            # row-sum of x via 64*attn row-sum matmul + precomputed hres sums
            psKD = h["psKD"]
            for ic in range(8):
                nc.tensor.matmul(psKD[:, 96:97], oTv[:, ic],
                                 t_woRS[:, ic:ic + 1],
                                 start=(ic == 0), stop=(ic == 7))
            xsum = cols_p.tile([128, 1], F32, tag="xsum", name="xsum")
            nc.vector.scalar_tensor_tensor(xsum[:], psKD[:, 96:97], 1.0 / 64.0,
                                           h["hr"][:, DM:DM + 1],
                                           OP.mult, OP.add)
            nmu = cols_p.tile([128, 1], F32, tag="nmu", name="nmu")
            nc.vector.tensor_scalar_mul(nmu[:], xsum[:], -1.0 / DM)
            var = cols_p.tile([128, 1], F32, tag="var", name="var")
            vscr = oc_p.tile([128, DM], BF16, tag="oc", name="vscr")
            nc.scalar.activation(vscr[:], x[:], AF.Square, bias=nmu[:],
                                 accum_out=var[:])
            vare = cols_p.tile([128, 1], F32, tag="vare", name="vare")
            nc.vector.tensor_scalar(vare[:], var[:], 1.0 / DM, float(LN_EPS),
                                    OP.mult, OP.add)
            sd = cols_p.tile([128, 1], F32, tag="sd", name="sd")
            nc.scalar.sqrt(sd[:], vare[:])
            rstd = cols_p.tile([128, 1], F32, tag="rstd", name="rstd")
            nc.vector.reciprocal(rstd[:], sd[:])
            nmr = cols_p.tile([128, 1], F32, tag="nmr", name="nmr")
            nc.vector.tensor_mul(nmr[:], nmu[:], rstd[:])
            xo = oc_p.tile([128, DM], BF16, tag="oc", name="xo")
            nc.scalar.activation(xo[:], x[:], AF.Identity, scale=rstd[:],
                                 bias=nmr[:])
            # issue the store from Act's DGE queue: it waits on xo anyway,
            # and this keeps the shared SP queue free for the G transposes
            nc.scalar.dma_start(d_out[cs, :], xo[:])

        load_consts()
        e = make_prepA(0)
        e(7)
        e = make_prepA(1)
        e(7)
        for hf in range(2):
            prepB1_half(0, hf)
            prepB2a_half(0, hf)
        prepB2a_fin(0)

        prepB2b(0)

        def emit_none(n):
            pass

        for c in range(n_chunks):
            emitA = make_prepA(c + 2) if c + 2 < n_chunks else emit_none
            scan_core(c, emitA, c + 1 < n_chunks)
            emitA(7)
            if c + 1 < n_chunks:
                prepB2b(c + 1)
            scan_tail(c)

    return nc


# ---------------------------------------------------------------- host side
_wq_cache = {}


def _prep_weights(W_qkvb, W_o):
    key = (W_qkvb.shape, float(W_qkvb.reshape(-1)[::4097].sum()),
           float(W_o.reshape(-1)[::2049].sum()))
    if _wq_cache.get("key") == key:
        return _wq_cache
    _wq_cache.clear()
    _wq_cache["key"] = key
    bf16, f8 = ml_dtypes.bfloat16, ml_dtypes.float8_e4m3
    Wr = W_qkvb.reshape(NH, 193, DM)
    wq = np.empty((DM, 3088), dtype=np.float32)
    wq[:, 0:1024] = Wr[:, 64:128, :].reshape(1024, DM).T           # K
    wq[:, 1024:2048] = Wr[:, 0:64, :].reshape(1024, DM).T          # Q
    wq[:, 2048:3072] = Wr[:, 128:192, :].reshape(1024, DM).T       # V
    wq[:, 3072:3088] = Wr[:, 192, :].T                             # b
    w8 = wq.astype(f8)
    dw8 = (wq - w8.astype(np.float32)).astype(f8)
    wo8 = (np.ascontiguousarray(W_o.T) * 64.0).astype(f8)          # [i, o]
    rs = wo8.astype(np.float32).sum(1)                             # [i]
    woRS = np.ascontiguousarray(rs.reshape(8, 128).T).astype(bf16)  # [p, ic]
    _wq_cache.update(w8=w8, dw8=dw8, wo8=wo8, woRS=woRS)
    return _wq_cache


def _prep_core_inputs(h_b, W_qkvb, W_o):
    bf16, f8 = ml_dtypes.bfloat16, ml_dtypes.float8_e4m3
    wc = _prep_weights(W_qkvb, W_o)
    hT = np.ascontiguousarray(h_b.T)                               # [1024, 2048]
    h8 = hT.astype(f8)
    dh8 = (hT - h8.astype(np.float32)).astype(f8)
    ii, jj = np.indices((128, 128))
    mUI = (jj >= ii).astype(bf16)
    mUS = (jj > ii).astype(bf16)
    hres = np.zeros((SLEN, DM + 8), dtype=bf16)
    hres[:, :DM] = h_b.astype(bf16)
    hres[:, DM] = h_b.astype(np.float32).sum(1).astype(bf16)
    return {"h8": h8, "dh8": dh8, "hres": hres,
            "w8": wc["w8"], "dw8": wc["dw8"], "wo8": wc["wo8"],
            "woRS": wc["woRS"], "maskUI": mUI, "maskUS": mUS}


_cached = {}


def kernel(h, W_qkvb, W_o, ln_g, ln_b):
    h = np.asarray(h, np.float32)
    W_qkvb = np.asarray(W_qkvb, np.float32)
    W_o = np.asarray(W_o, np.float32)
    ln_g = np.asarray(ln_g, np.float32)
    ln_b = np.asarray(ln_b, np.float32)
    if "nc" not in _cached:
        _cached["nc"] = build_program()
    nc = _cached["nc"]
    in_maps = [_prep_core_inputs(h[:, b, :], W_qkvb, W_o)
               for b in range(BSZ)]
    res = run_bass_kernel_spmd(nc, in_maps, list(range(BSZ)),
                               trace=os.environ.get("BASS_TRACE", "") == "1")
    out = np.stack([res.results[b]["out"] for b in range(BSZ)], axis=1)
    kernel.last_exec_time_ns = res.exec_time_ns
    return out.astype(np.float32) * ln_g + ln_b


# revision 74
# speedup vs baseline: 1.0704x; 1.0704x over previous
"""Trainium2 Bass kernel for the CudaNorm FastWeight DPFP transformer layer.

Sharding: batch (8) across the 8 cores; each core runs its batch's full layer.

v5: software pipeline reordered so the mask/gram chain (prepB2a) for chunk c+1
runs a full period ahead of its consumer (prepB2b/gram_back), per-half
roll->transpose interleave with split G tiles, W-update in 4x4-head groups
with split W tiles, psKD double-buffered, masks split Pool/DVE, unnormalized-K
trick, PE ones-matmul feature sums, LN center/scale on Act (ln_g/ln_b on
host).
"""
import os
import numpy as np
import ml_dtypes

import concourse.bass as bass
import concourse.mybir as mybir
from concourse.bass_utils import run_bass_kernel_spmd
from concourse.tile import TileContext
from concourse.vector_clock import ScopedClock, VectorClock
from contextlib import ExitStack

F32 = mybir.dt.float32
BF16 = mybir.dt.bfloat16
F8 = mybir.dt.float8e4
DR = mybir.MatmulPerfMode.DoubleRow
AF = mybir.ActivationFunctionType
OP = mybir.AluOpType
AX = mybir.AxisListType

SLEN, BSZ, DM = 2048, 8, 1024
NH, DH, NROLL = 16, 64, 2
D = 2 * NROLL * DH            # 256 feature dim
C = 128                       # chunk length
NCH = SLEN // C               # 16 chunks
EPS, LN_EPS = 1e-5, 1e-5
SCALE = 1.0 / float(np.sqrt(DH))
NHORN = 1                     # Horner/Neumann solve iterations
MASK_DVE = {6, 7}             # gram pr indices whose masks run on DVE

# ---------------------------------------------------------------- tile ctx
MAXW = 2


class PatchedTileContext(TileContext):
    """Work around walrus TPB sync-command limits: each instruction carries at
    most 2 sync commands (waits+updates); hoist excess waits onto preceding
    same-engine NoOps (1 wait each), and emit the kernel-tail drain's waits
    one-per-nop on SP."""

    def _lower_ordered_insts(self, ordered):
        for bb_name in list(ordered.keys()):
            new = []
            for inst in ordered[bb_name]:
                si = inst.sync_info
                nupd = len(si.on_update) if si is not None and si.on_update else 0
                maxw = max(0, MAXW - nupd)
                if si is not None and si.on_wait and len(si.on_wait) > maxw:
                    waits = list(si.on_wait)
                    excess = waits if maxw == 0 else waits[:-maxw]
                    keep = [] if maxw == 0 else waits[-maxw:]
                    for w in excess:
                        nop = mybir.InstNoOp(
                            name=self.nc.get_next_instruction_name(),
                            engine=inst.engine, ins=[], outs=[])
                        nop.sync_info = mybir.SyncInfo(on_wait=[w], on_update=[])
                        new.append(nop)
                    inst.sync_info = mybir.SyncInfo(
                        on_wait=keep, on_update=list(si.on_update or []))
                new.append(inst)
            ordered[bb_name] = new
        return super()._lower_ordered_insts(ordered)

    def _drain_and_barrier(self, tick_clock, wait_clock):
        gc = tick_clock.global_clock
        n = len(gc)
        for p in range(n):
            if gc[p] > 0:
                vc = VectorClock([gc[i] if i == p else 0 for i in range(n)])
                nop = self.nc.sync.nop(nofuse=True)
                wait_clock.add_sem_waits(nop.ins, ScopedClock({None: vc}))
        self.nc.sync.drain()
        self.nc.all_engine_barrier()
        assert self.sems is not None
        popped = self.nc._tile_sem_poison_stack.pop()
        assert popped is self._sem_poison
        self.nc.clear_and_free_semaphores(list(self.sems.allocated().values()))
        self.nc.all_engine_barrier()


# ---------------------------------------------------------------- program
def build_program(n_chunks=NCH):
    nc = bass.Bass()
    d_h8 = nc.declare_dram_parameter("h8", [DM, SLEN], F8, isOutput=False)
    d_dh8 = nc.declare_dram_parameter("dh8", [DM, SLEN], F8, isOutput=False)
    d_hres = nc.declare_dram_parameter("hres", [SLEN, DM + 8], BF16,
                                       isOutput=False)
    d_w8 = nc.declare_dram_parameter("w8", [DM, 3088], F8, isOutput=False)
    d_dw8 = nc.declare_dram_parameter("dw8", [DM, 3088], F8, isOutput=False)
    d_wo8 = nc.declare_dram_parameter("wo8", [DM, DM], F8, isOutput=False)
    d_woRS = nc.declare_dram_parameter("woRS", [128, 8], BF16, isOutput=False)
    d_mUI = nc.declare_dram_parameter("maskUI", [128, 128], BF16, isOutput=False)
    d_mUS = nc.declare_dram_parameter("maskUS", [128, 128], BF16, isOutput=False)
    d_out = nc.declare_dram_parameter("out", [SLEN, DM], BF16, isOutput=True)

    with PatchedTileContext(nc) as tc, ExitStack() as ctx:
        P = lambda name, bufs, **kw: ctx.enter_context(
            tc.tile_pool(name=name, bufs=bufs, **kw))
        const = P("const", 1)
        state = P("state", 1)
        hts_p = P("hts", 2)
        xp_p = P("xp", 2)
        f_p = P("f", 2)
        G_p = P("G", 2)
        V_p = P("V", 2)
        t1_p = P("t1", 2)
        cols_p = P("cols", 3)
        sa_p = P("sa", 3)
        sh_p = P("sh", 2)
        nt_p = P("nt", 2)
        mg_p = P("mg", 3)
        oc_p = P("oc", 2)
        oT_p = P("oT", 1)
        hr_p = P("hr", 2)
        x_p = P("x", 2)
        psP_p = P("psP", 2, space="PSUM")
        psG_p = P("psG", 2, space="PSUM")
        psS_p = P("psS", 2, space="PSUM")
        psK_p = P("psK", 2, space="PSUM")

        # ---- constants
        t_mUI = const.tile([128, 128], BF16, tag="mUI", name="mUI")
        t_mUS = const.tile([128, 128], BF16, tag="mUS", name="mUS")
        t_ones = const.tile([128, 1], BF16, tag="ones", name="ones")
        nc.vector.memset(t_ones[:], 1.0)
        # fp8 DoubleRow weight tiles: [128, 2 dm-blocks x 3088]
        t_w8 = [const.tile([128, 2 * 3088], F8, tag=f"w8{m}", name=f"w8{m}")
                for m in range(4)]
        t_dw8 = [const.tile([128, 2 * 3088], F8, tag=f"dw8{m}", name=f"dw8{m}")
                 for m in range(4)]
        # Wo in fp8 pair-interleaved DoubleRow layout (values are 64*W_o);
        # row (m, p, e) holds W_o[:, 256m + 2p + e]
        t_wo8 = [const.tile([128, 2 * DM], F8, tag=f"wo8{m}", name=f"wo8{m}")
                 for m in range(4)]
        t_woRS = const.tile([128, 8], BF16, tag="woRS", name="woRS")

        def load_consts():
            for m in range(4):
                for e in range(2):
                    mc = 2 * m + e
                    nc.sync.dma_start(
                        t_w8[m][:, e * 3088:(e + 1) * 3088],
                        d_w8[mc * 128:(mc + 1) * 128, :])
                    nc.sync.dma_start(
                        t_dw8[m][:, e * 3088:(e + 1) * 3088],
                        d_dw8[mc * 128:(mc + 1) * 128, :])
            nc.sync.dma_start(t_mUI[:], d_mUI[:])
            nc.sync.dma_start(t_mUS[:], d_mUS[:])
            nc.sync.dma_start(t_woRS[:], d_woRS[:])
            for m in range(4):
                for e in range(2):
                    ic = 2 * m + e
                    nc.sync.dma_start(t_wo8[m][:, e * DM:(e + 1) * DM],
                                      d_wo8[ic * 128:(ic + 1) * 128, :])

        # ---- state (W accumulates directly in bf16; increments are bf16
        # products so fp32 mastering gains nothing - verified in proto)
        t_Wh = [state.tile([128, 8 * 128], BF16, tag=f"Wb{hf}", name=f"Wb{hf}")
                for hf in range(2)]
        for hf in range(2):
            nc.vector.memset(t_Wh[hf][:], 0.0)
        t_r32 = state.tile([128, 32], F32, tag="r32", name="r32")
        nc.vector.memset(t_r32[:], 0.0)
        t_rb = state.tile([128, 32], BF16, tag="rb", name="rb")
        nc.vector.memset(t_rb[:], 0.0)
        t_Y = state.tile([128, NH * DH], BF16, tag="Y", name="Y")

        def Wb(hd):
            return t_Wh[hd // 8], (hd % 8) * 128

        H = {}

        def prepA_dma(c):
            cs = slice(c * 128, (c + 1) * 128)
            hts = hts_p.tile([128, 1024], F8, tag="hts", name="hts")
            nc.sync.dma_start(
                hts[:].rearrange("p (mc t) -> p mc t", t=128),
                d_h8[:, cs].rearrange("(mc p) t -> p mc t", p=128))
            dhts = hts_p.tile([128, 1024], F8, tag="dhts", name="dhts")
            nc.sync.dma_start(
                dhts[:].rearrange("p (mc t) -> p mc t", t=128),
                d_dh8[:, cs].rearrange("(mc p) t -> p mc t", p=128))
            hr = hr_p.tile([128, DM + 8], BF16, tag="hr", name="hr")
            nc.sync.dma_start(hr[:], d_hres[cs, :])
            return hts, dhts, hr

        def proj_qkvb(ps, hts, dhts, lo, hi):
            """ps += h @ W_qkvb[:, lo:hi] via 3-term fp8 DoubleRow matmuls."""
            hv = hts[:].rearrange("p (mc t) -> p mc t", t=128)
            dhv = dhts[:].rearrange("p (mc t) -> p mc t", t=128)
            k, terms = 0, [(hv, t_w8), (hv, t_dw8), (dhv, t_w8)]
            for ht, tw in terms:
                for m in range(4):
                    wv = tw[m][:].rearrange("p (e w) -> p e w", w=3088)
                    nc.tensor.matmul(ps[:], ht[:, 2 * m:2 * m + 2, :],
                                     wv[:, :, lo:hi],
                                     start=(k == 0), stop=(k == 11),
                                     perf_mode=DR)
                    k += 1

        def make_prepA(c):
            """Incremental projection emitter: emit(n) lays down n og-groups
            (12 DR matmuls + activation each); og 6 is the sigmoid b block."""
            hts, dhts, hr = prepA_dma(c)
            xpC = xp_p.tile([128, 4096], BF16, tag="xpC", name="xpC")
            xr = xpC[:].rearrange("p (h w) -> p h w", w=256)
            Vall = V_p.tile([128, 1024], BF16, tag="V", name="V")
            sig = cols_p.tile([128, NH], F32, tag="sig", name="sig")
            H[c] = dict(xpC=xpC, Vall=Vall, sig=sig, hr=hr)
            st = {"og": 0}

            def emit(n):
                for _ in range(n):
                    og = st["og"]
                    if og >= 7:
                        return
                    st["og"] += 1
                    if og < 6:
                        ps = psP_p.tile([128, 512], F32, tag="pp", name="pp")
                        proj_qkvb(ps, hts, dhts, og * 512, (og + 1) * 512)
                        psr = ps[:].rearrange("p (h v) -> p h v", v=64)
                        if og < 4:  # K (og 0,1) / Q (og 2,3)
                            o = 128 if og < 2 else 0
                            hs = slice((og % 2) * 8, (og % 2) * 8 + 8)
                            nc.scalar.activation(xr[:, hs, o:o + 64], psr,
                                                 AF.Relu)
                            nc.scalar.activation(xr[:, hs, o + 64:o + 128],
                                                 psr, AF.Relu, scale=-1.0)
                        else:       # V
                            gv = og - 4
                            nc.scalar.copy(Vall[:, gv * 512:(gv + 1) * 512],
                                           ps[:])
                    else:
                        psb = psP_p.tile([128, NH], F32, tag="pp", name="pp")
                        proj_qkvb(psb, hts, dhts, 3072, 3088)
                        nc.scalar.activation(sig[:], psb[:], AF.Sigmoid)
            return emit

        def prepB1_half(c, hf):
            h = H[c]
            if hf == 0:
                # f tiles: per dc, per head [Q-roll 128 | K-roll 128]; K raw
                h["f"] = [f_p.tile([128, 4096], BF16, tag=f"f{dc}",
                                   name=f"f{dc}") for dc in range(2)]
                # G[dc][hf]: 16 blocks [128f x 128t] for heads hf*8..hf*8+7
                h["G"] = [[G_p.tile([128, 2048], BF16, tag=f"G{dc}{g}",
                                    name=f"G{dc}{g}") for g in range(2)]
                          for dc in range(2)]
            f, G = h["f"], h["G"]
            fr = [f[dc][:].rearrange("p (h w) -> p h w", w=256)
                  for dc in range(2)]
            xr = h["xpC"][:].rearrange("p (h w) -> p h w", w=256)
            hs = slice(hf * 8, hf * 8 + 8)
            for dc in range(2):
                rl = dc + 1
                for o in (0, 128):  # Q block, K block per head
                    nc.vector.tensor_mul(fr[dc][:, hs, o + rl:o + 128],
                                         xr[:, hs, o + rl:o + 128],
                                         xr[:, hs, o:o + 128 - rl])
                    nc.vector.tensor_mul(fr[dc][:, hs, o:o + rl],
                                         xr[:, hs, o:o + rl],
                                         xr[:, hs, o + 128 - rl:o + 128])
            for dc in range(2):
                nc.sync.dma_start_transpose(
                    G[dc][hf][:].rearrange("p (b l) -> p b l", l=128),
                    f[dc][:, hf * 2048:(hf + 1) * 2048])

        def Gb(h, dc, hd):
            """(tile, base offset) of head hd's [Q128|K128] block pair."""
            return h["G"][dc][hd // 8], (hd % 8) * 256

        def prepB2a_half(c, hf):
            h = H[c]
            if hf == 0:
                # psKD bank: [kd 16|dn 16|sumQ 16|sumK 16|rcols 32|xsum 1]
                h["psKD"] = psK_p.tile([128, 97], F32, tag="kdps", name="kdps")
                h["Shat"] = sh_p.tile([128, NH * 128], BF16, tag="sh",
                                      name="sh")
                h["NT"] = nt_p.tile([128, NH * 128], BF16, tag="nt", name="nt")
            psKD, Shat, NT = h["psKD"], h["Shat"], h["NT"]

            def gram_front(pr):
                ps = psG_p.tile([128, 512], F32, tag="gg", name="gg")
                for e in range(2):
                    hd = pr * 2 + e
                    Gt0, b0 = Gb(h, 0, hd)
                    Gt1, b1 = Gb(h, 1, hd)
                    nc.tensor.matmul(ps[:, e * 256:e * 256 + 256],
                                     Gt0[:, b0 + 128:b0 + 256],
                                     Gt0[:, b0:b0 + 256],
                                     start=True, stop=False)
                    nc.tensor.matmul(ps[:, e * 256:e * 256 + 256],
                                     Gt1[:, b1 + 128:b1 + 256],
                                     Gt1[:, b1:b1 + 256],
                                     start=False, stop=True)
                sa = sa_p.tile([128, 512], BF16, tag="sa", name="sa")
                nc.scalar.copy(sa[:], ps[:])
                sar = sa[:].rearrange("p (e w) -> p e w", w=256)
                eng = nc.vector if pr in MASK_DVE else nc.gpsimd
                eng.tensor_mul(
                    Shat[:, pr * 256:(pr + 1) * 256].rearrange(
                        "p (e w) -> p e w", w=128),
                    sar[:, :, 0:128],
                    t_mUI[:].unsqueeze(1).broadcast_to((128, 2, 128)))
                eng.tensor_mul(
                    NT[:, pr * 256:(pr + 1) * 256].rearrange(
                        "p (e w) -> p e w", w=128),
                    sar[:, :, 128:256],
                    t_mUS[:].unsqueeze(1).broadcast_to((128, 2, 128)))

            # feature sums via PE ones-matmuls on raw G blocks
            for hd in range(hf * 8, hf * 8 + 8):
                for dc in range(2):
                    Gt, b = Gb(h, dc, hd)
                    nc.tensor.matmul(psKD[:, 32 + hd:33 + hd],
                                     Gt[:, b:b + 128], t_ones[:],
                                     start=(dc == 0), stop=(dc == 1))
                for dc in range(2):
                    Gt, b = Gb(h, dc, hd)
                    nc.tensor.matmul(psKD[:, 48 + hd:49 + hd],
                                     Gt[:, b + 128:b + 256], t_ones[:],
                                     start=(dc == 0), stop=(dc == 1))
            for pr in range(hf * 4, hf * 4 + 4):
                gram_front(pr)

        def prepB2a_fin(c):
            h = H[c]
            psKD = h["psKD"]
            frecQ = cols_p.tile([128, NH], F32, tag="frecQ", name="frecQ")
            nc.vector.reciprocal(frecQ[:], psKD[:, 32:48])
            cKf = cols_p.tile([128, NH], F32, tag="cKf", name="cKf")
            nc.vector.reciprocal(cKf[:], psKD[:, 48:64])
            cKb = cols_p.tile([128, NH], BF16, tag="cKb", name="cKb")
            nc.scalar.copy(cKb[:], cKf[:])
            cK2 = cols_p.tile([128, NH], F32, tag="cK2", name="cK2")
            nc.vector.tensor_mul(cK2[:], cKf[:], cKf[:])
            fQS = cols_p.tile([128, NH], F32, tag="fQS", name="fQS")
            nc.vector.tensor_scalar_mul(fQS[:], frecQ[:], SCALE)
            h.update(frecQ=frecQ, cKf=cKf, cKb=cKb, cK2=cK2, fQS=fQS)

        def prepB2b(c):
            h = H[c]
            psKD, cKb, cKf, frecQ = h["psKD"], h["cKb"], h["cKf"], h["frecQ"]
            Shat, NT = h["Shat"], h["NT"]

            def gram_back(pr):
                for e in range(2):
                    hd = pr * 2 + e
                    for dc in range(2):
                        Gt, b = Gb(h, dc, hd)
                        nc.tensor.matmul(psKD[:, hd:hd + 1],
                                         Gt[:, b + 128:b + 256],
                                         t_rb[:, dc * 16 + hd:dc * 16 + hd + 1],
                                         start=(dc == 0), stop=False)
                    nc.tensor.matmul(psKD[:, hd:hd + 1],
                                     NT[:, hd * 128:(hd + 1) * 128],
                                     cKb[:, hd:hd + 1], start=False, stop=True)
                    for dc in range(2):
                        Gt, b = Gb(h, dc, hd)
                        nc.tensor.matmul(psKD[:, 16 + hd:17 + hd],
                                         Gt[:, b:b + 128],
                                         t_rb[:, dc * 16 + hd:dc * 16 + hd + 1],
                                         start=(dc == 0), stop=False)
                    nc.tensor.matmul(psKD[:, 16 + hd:17 + hd],
                                     Shat[:, hd * 128:(hd + 1) * 128],
                                     cKb[:, hd:hd + 1], start=False, stop=True)

            for pr in range(NH // 2):
                gram_back(pr)
            kd = cols_p.tile([128, NH], F32, tag="kd", name="kd")
            nc.vector.tensor_mul(kd[:], psKD[:, 0:16], cKf[:])
            if c == 0:
                nc.vector.memset(kd[0:1, :], 1.0)

            # chunk-level columns
            ceps = cols_p.tile([128, NH], F32, tag="ceps", name="ceps")
            nc.vector.tensor_scalar_add(ceps[:], kd[:], EPS)
            cc = cols_p.tile([128, NH], F32, tag="cc", name="cc")
            nc.vector.reciprocal(cc[:], ceps[:])
            t0 = cols_p.tile([128, NH], F32, tag="t0", name="t0")
            nc.vector.tensor_mul(t0[:], kd[:], cc[:])
            cb = cols_p.tile([128, NH], F32, tag="cb", name="cb")
            nc.vector.tensor_mul(cb[:], t0[:], h["sig"][:])
            cbK = cols_p.tile([128, NH], F32, tag="cbK", name="cbK")
            nc.vector.tensor_mul(cbK[:], cb[:], cKf[:])
            cbc = cols_p.tile([128, NH], F32, tag="cbc", name="cbc")
            nc.vector.tensor_mul(cbc[:], cb[:], cc[:])
            cbc2 = cols_p.tile([128, NH], F32, tag="cbc2", name="cbc2")
            nc.vector.tensor_mul(cbc2[:], cbc[:], h["cK2"][:])
            dnm = cols_p.tile([128, NH], F32, tag="dnm", name="dnm")
            nc.vector.tensor_mul(dnm[:], psKD[:, 16:32], frecQ[:])
            dne = cols_p.tile([128, NH], F32, tag="dne", name="dne")
            nc.vector.tensor_scalar_add(dne[:], dnm[:], EPS)
            dnr = cols_p.tile([128, NH], F32, tag="dnr", name="dnr")
            nc.vector.reciprocal(dnr[:], dne[:])
            dnrS = cols_p.tile([128, NH], F32, tag="dnrS", name="dnrS")
            nc.vector.tensor_mul(dnrS[:], dnr[:], h["fQS"][:])
            # t1-hat = (cb*cK) * V, broadcast mul on Pool (SBUF-only operands)
            t1 = t1_p.tile([128, 1024], BF16, tag="t1", name="t1")
            nc.gpsimd.tensor_mul(
                t1[:].rearrange("p (h v) -> p h v", v=64),
                h["Vall"][:].rearrange("p (h v) -> p h v", v=64),
                cbK[:].unsqueeze(2).broadcast_to((128, NH, 64)))
            h.update(cbc2=cbc2, dnrS=dnrS, t1=t1)

        def bscale(out, ps, colvec, g):
            """out (bf16) = psum [128,512] * per-head column broadcast."""
            nc.vector.tensor_tensor(
                out.rearrange("p (h v) -> p h v", v=64),
                ps[:].rearrange("p (h v) -> p h v", v=64),
                colvec[:, g * 8:(g + 1) * 8].unsqueeze(2).broadcast_to((128, 8, 64)),
                OP.mult)

        def scan_core(c, emitA, nxt):
            h = H[c]
            f, cbc2, dnrS = h["f"], h["cbc2"], h["dnrS"]
            cKb, psKD = h["cKb"], h["psKD"]
            emitA(2)
            if nxt:
                prepB1_half(c + 1, 0)
            # ---- KW + X0 = t1 - cbc2*KW (in place over t1), subs on Pool
            X0 = h["t1"]
            for g in range(2):
                ps = psS_p.tile([128, 512], F32, tag="ss", name="ss")
                for j in range(8):
                    hd = g * 8 + j
                    Gt, b = Gb(h, 0, hd)
                    Gt1, b1 = Gb(h, 1, hd)
                    Wt, wb = Wb(hd)
                    nc.tensor.matmul(ps[:, j * 64:(j + 1) * 64],
                                     Gt[:, b + 128:b + 256],
                                     Wt[:, wb:wb + 64],
                                     start=True, stop=False)
                    nc.tensor.matmul(ps[:, j * 64:(j + 1) * 64],
                                     Gt1[:, b1 + 128:b1 + 256],
                                     Wt[:, wb + 64:wb + 128],
                                     start=False, stop=True)
                mg = mg_p.tile([128, 512], BF16, tag="mg", name="mg")
                bscale(mg[:], ps, cbc2, g)
                nc.gpsimd.tensor_sub(X0[:, g * 512:(g + 1) * 512],
                                     X0[:, g * 512:(g + 1) * 512], mg[:])
            emitA(1)
            if nxt:
                prepB1_half(c + 1, 1)
            # ---- Horner iterations: Z <- X0 - cbc2*(NT^T Z); last writes t_Y
            Zin = X0
            for it in range(NHORN):
                for g in range(2):
                    ps = psS_p.tile([128, 512], F32, tag="ss", name="ss")
                    for j in range(8):
                        hd = g * 8 + j
                        nc.tensor.matmul(ps[:, j * 64:(j + 1) * 64],
                                         h["NT"][:, hd * 128:(hd + 1) * 128],
                                         Zin[:, hd * 64:(hd + 1) * 64],
                                         start=True, stop=True)
                    mg = mg_p.tile([128, 512], BF16, tag="mg", name="mg")
                    bscale(mg[:], ps, cbc2, g)
                    nc.gpsimd.tensor_sub(t_Y[:, g * 512:(g + 1) * 512],
                                         X0[:, g * 512:(g + 1) * 512], mg[:])
                Zin = t_Y
            emitA(1)
            # ---- outc = (QW + tril_incl(S1) Yhat) * dnrS
            outc = oc_p.tile([128, 1024], BF16, tag="oc", name="oc")
            for g in range(2):
                ps = psS_p.tile([128, 512], F32, tag="ss", name="ss")
                for j in range(8):
                    hd = g * 8 + j
                    Wt, wb = Wb(hd)
                    for dc in range(2):
                        Gt, b = Gb(h, dc, hd)
                        nc.tensor.matmul(ps[:, j * 64:(j + 1) * 64],
                                         Gt[:, b:b + 128],
                                         Wt[:, wb + dc * 64:wb + dc * 64 + 64],
                                         start=(dc == 0), stop=False)
                    nc.tensor.matmul(ps[:, j * 64:(j + 1) * 64],
                                     h["Shat"][:, hd * 128:(hd + 1) * 128],
                                     t_Y[:, hd * 64:(hd + 1) * 64],
                                     start=False, stop=True)
                bscale(outc[:, g * 512:(g + 1) * 512], ps, dnrS, g)
            # ---- transpose outc -> outT [i, t] via one DMA, then fp8 cast
            outT = oT_p.tile([128, 1024], BF16, tag="oT", name="oT")
            nc.sync.dma_start_transpose(
                outT[:].rearrange("p (b l) -> p b l", l=128), outc[:])
            oT8 = oT_p.tile([128, 1024], F8, tag="oT8", name="oT8")
            nc.scalar.copy(oT8[:], outT[:])
            h["outT"] = oT8
            emitA(2)
            if nxt:
                prepB2a_half(c + 1, 0)
            # ---- W state update: Wm += Kraw^T Yhat; r += Kraw^T cK
            for q in range(4):
                hds = range(q * 4, q * 4 + 4)
                ps = psS_p.tile([128, 512], F32, tag="ss", name="ss")
                for j, hd in enumerate(hds):
                    for dc in range(2):
                        nc.tensor.matmul(
                            ps[:, j * 128 + dc * 64:j * 128 + dc * 64 + 64],
                            f[dc][:, hd * 256 + 128:hd * 256 + 256],
                            t_Y[:, hd * 64:(hd + 1) * 64],
                            start=True, stop=True)
                        nc.tensor.matmul(
                            psKD[:, 64 + dc * 16 + hd:65 + dc * 16 + hd],
                            f[dc][:, hd * 256 + 128:hd * 256 + 256],
                            cKb[:, hd:hd + 1], start=True, stop=True)
                Wt, wb = Wb(q * 4)
                nc.vector.tensor_add(Wt[:, wb:wb + 512], ps[:],
                                     Wt[:, wb:wb + 512])
            nc.vector.tensor_add(t_r32[:], psKD[:, 64:96], t_r32[:])
            nc.scalar.copy(t_rb[:], t_r32[:])
            emitA(1)
            if nxt:
                prepB2a_half(c + 1, 1)
                prepB2a_fin(c + 1)

        def scan_tail(c):
            cs = slice(c * 128, (c + 1) * 128)
            h = H.pop(c)
            outT = h["outT"]
            # ---- output projection + residual + LN (center/scale via Act)
            x = x_p.tile([128, DM], BF16, tag="x", name="x")
            oTv = outT[:].rearrange("p (ic t) -> p ic t", t=128)
            for og in range(2):
                ps = psP_p.tile([128, 512], F32, tag="pp", name="pp")
                for m in range(4):
                    wv = t_wo8[m][:].rearrange("p (e w) -> p e w", w=DM)
                    nc.tensor.matmul(ps[:], oTv[:, 2 * m:2 * m + 2, :],
                                     wv[:, :, og * 512:(og + 1) * 512],
                                     start=(m == 0), stop=(m == 3),
                                     perf_mode=DR)
                # psum holds 64*attn (wo8 stores 64*W_o); rescale in the add
                nc.vector.scalar_tensor_tensor(
                    x[:, og * 512:(og + 1) * 512], ps[:], 1.0 / 64.0,
                    h["hr"][:, og * 512:(og + 1) * 512], OP.mult, OP.add)
            # row-sum of x via 64*attn row-sum matmul + precomputed hres sums
            psKD = h["psKD"]
            for ic in range(8):
                nc.tensor.matmul(psKD[:, 96:97], oTv[:, ic],
                                 t_woRS[:, ic:ic + 1],
                                 start=(ic == 0), stop=(ic == 7))
            xsum = cols_p.tile([128, 1], F32, tag="xsum", name="xsum")
            nc.vector.scalar_tensor_tensor(xsum[:], psKD[:, 96:97], 1.0 / 64.0,
                                           h["hr"][:, DM:DM + 1],
                                           OP.mult, OP.add)
            nmu = cols_p.tile([128, 1], F32, tag="nmu", name="nmu")
            nc.vector.tensor_scalar_mul(nmu[:], xsum[:], -1.0 / DM)
            var = cols_p.tile([128, 1], F32, tag="var", name="var")
            vscr = oc_p.tile([128, DM], BF16, tag="oc", name="vscr")
            nc.scalar.activation(vscr[:], x[:], AF.Square, bias=nmu[:],
                                 accum_out=var[:])
            vare = cols_p.tile([128, 1], F32, tag="vare", name="vare")
            nc.vector.tensor_scalar(vare[:], var[:], 1.0 / DM, float(LN_EPS),
                                    OP.mult, OP.add)
            sd = cols_p.tile([128, 1], F32, tag="sd", name="sd")
            nc.scalar.sqrt(sd[:], vare[:])
            rstd = cols_p.tile([128, 1], F32, tag="rstd", name="rstd")
            nc.vector.reciprocal(rstd[:], sd[:])
            nmr = cols_p.tile([128, 1], F32, tag="nmr", name="nmr")
            nc.vector.tensor_mul(nmr[:], nmu[:], rstd[:])
            xo = oc_p.tile([128, DM], BF16, tag="oc", name="xo")
            nc.scalar.activation(xo[:], x[:], AF.Identity, scale=rstd[:],
                                 bias=nmr[:])
            # issue the store from Act's DGE queue: it waits on xo anyway,
            # and this keeps the shared SP queue free for the G transposes
            nc.scalar.dma_start(d_out[cs, :], xo[:])

        load_consts()
        e = make_prepA(0)
        e(7)
        e = make_prepA(1)
        e(7)
        for hf in range(2):
            prepB1_half(0, hf)
            prepB2a_half(0, hf)
        prepB2a_fin(0)

        prepB2b(0)

        def emit_none(n):
            pass

        for c in range(n_chunks):
            emitA = make_prepA(c + 2) if c + 2 < n_chunks else emit_none
            scan_core(c, emitA, c + 1 < n_chunks)
            emitA(7)
            if c + 1 < n_chunks:
                prepB2b(c + 1)
            scan_tail(c)

    return nc


# ---------------------------------------------------------------- host side
_wq_cache = {}


def _prep_weights(W_qkvb, W_o):
    key = (W_qkvb.shape, float(W_qkvb.reshape(-1)[::4097].sum()),
           float(W_o.reshape(-1)[::2049].sum()))
    if _wq_cache.get("key") == key:
        return _wq_cache
    _wq_cache.clear()
    _wq_cache["key"] = key
    bf16, f8 = ml_dtypes.bfloat16, ml_dtypes.float8_e4m3
    Wr = W_qkvb.reshape(NH, 193, DM)
    wq = np.empty((DM, 3088), dtype=np.float32)
    wq[:, 0:1024] = Wr[:, 64:128, :].reshape(1024, DM).T           # K
    wq[:, 1024:2048] = Wr[:, 0:64, :].reshape(1024, DM).T          # Q
    wq[:, 2048:3072] = Wr[:, 128:192, :].reshape(1024, DM).T       # V
    wq[:, 3072:3088] = Wr[:, 192, :].T                             # b
    w8 = wq.astype(f8)
    dw8 = (wq - w8.astype(np.float32)).astype(f8)
    wo8 = (np.ascontiguousarray(W_o.T) * 64.0).astype(f8)          # [i, o]
    rs = wo8.astype(np.float32).sum(1)                             # [i]
    woRS = np.ascontiguousarray(rs.reshape(8, 128).T).astype(bf16)  # [p, ic]
    _wq_cache.update(w8=w8, dw8=dw8, wo8=wo8, woRS=woRS)
    return _wq_cache


def _prep_core_inputs(h_b, W_qkvb, W_o):
    bf16, f8 = ml_dtypes.bfloat16, ml_dtypes.float8_e4m3
    wc = _prep_weights(W_qkvb, W_o)
    hT = np.ascontiguousarray(h_b.T)                               # [1024, 2048]
    h8 = hT.astype(f8)
    dh8 = (hT - h8.astype(np.float32)).astype(f8)
    ii, jj = np.indices((128, 128))
    mUI = (jj >= ii).astype(bf16)
    mUS = (jj > ii).astype(bf16)
    hres = np.zeros((SLEN, DM + 8), dtype=bf16)
    hres[:, :DM] = h_b.astype(bf16)
    hres[:, DM] = h_b.astype(np.float32).sum(1).astype(bf16)
    return {"h8": h8, "dh8": dh8, "hres": hres,
            "w8": wc["w8"], "dw8": wc["dw8"], "wo8": wc["wo8"],
            "woRS": wc["woRS"], "maskUI": mUI, "maskUS": mUS}


_cached = {}


def kernel(h, W_qkvb, W_o, ln_g, ln_b):
    h = np.asarray(h, np.float32)
    W_qkvb = np.asarray(W_qkvb, np.float32)
    W_o = np.asarray(W_o, np.float32)
    ln_g = np.asarray(ln_g, np.float32)
    ln_b = np.asarray(ln_b, np.float32)
    if "nc" not in _cached:
        _cached["nc"] = build_program()
    nc = _cached["nc"]
    in_maps = [_prep_core_inputs(h[:, b, :], W_qkvb, W_o)
               for b in range(BSZ)]
    res = run_bass_kernel_spmd(nc, in_maps, list(range(BSZ)),
                               trace=os.environ.get("BASS_TRACE", "") == "1")
    out = np.stack([res.results[b]["out"] for b in range(BSZ)], axis=1)
    kernel.last_exec_time_ns = res.exec_time_ns
    return out.astype(np.float32) * ln_g + ln_b
